# revision 1
# baseline (speedup 1.0000x reference)
"""Causal self-attention kernel for 8 TRN2 NeuronCores.

Problem (hardcoded): B=2, T=4096, C=768, NH=12, HS=64.
  qkv = x @ w_attn + b_attn; per-head causal softmax attention;
  y = att_out @ w_proj + b_proj

Sharding: 24 (batch, head) units over 8 cores -> 3 heads per core.
  cores 0..3: batch 0, heads (0,1,2), (3,4,5), (6,7,8), (9,10,11)
  cores 4..7: batch 1, same head split.
Each core computes a partial y^T [C, T]; the host sums partials per batch
and adds b_proj. The host also pre-transposes x (x^T is pure input
marshalling), so the kernel loads x^T chunks directly.

Per-core dataflow (f32r matmuls, fp32 PSUM accumulation). The attention
inner loop is software-pipelined one k-tile ahead (S(u+1) issued before
AV(u)) so the PE never stalls on the ScalarE exp. Attention accumulators
are evacuated from PSUM to SBUF immediately after their last AV, the
softmax denominators go through reciprocal_approx_fast (single DVE op),
and the next t-block's QKV matmuls are interleaved into the normalize/
projection epilogue so the PE stays busy across phase boundaries.
"""

import numpy as np

B, T, C, NH = 2, 4096, 768, 12
HS = C // NH          # 64
NCORES = 8
HPC = 3               # heads per core
QB = 512              # q block (moving dim)
NQB = T // QB         # 8
NKT = T // 128        # 32 k-tiles
NTB = T // QB         # t-blocks
NCC = C // 128        # 6 contraction chunks
VP_W = 128            # V'' width per k-tile (64 v + ones + zero pad, FWL)
WQJ = 7 * 128   # 896

_CACHE = {}


def _build():
    import contextlib
    import concourse.bacc as bacc
    import concourse.mybir as mybir
    from concourse.tile import TileContext
    from concourse.masks import make_identity

    f32 = mybir.dt.float32
    f32r = mybir.dt.float32r
    bf16 = mybir.dt.bfloat16
    Exp = mybir.ActivationFunctionType.Exp
    Ident = mybir.ActivationFunctionType.Identity
    mult = mybir.AluOpType.mult

    nc = bacc.Bacc(trn_type="TRN2")

    xt_d = nc.dram_tensor("xt", [C, T], f32, kind="ExternalInput")
    wq = nc.dram_tensor("wq", [C, WQJ], f32, kind="ExternalInput")
    bq = nc.dram_tensor("bq", [128, 7], f32, kind="ExternalInput")
    wp = nc.dram_tensor("wp", [192, C], f32, kind="ExternalInput")
    y = nc.dram_tensor("y", [C, T], f32, kind="ExternalOutput")

    # j-blocks: 0:[qA|qB] 1:[kA|kB] 2:[qC|qC] 3:[kC|kC] 4:vA' 5:vB' 6:vC'
    JBLK = [(0, 128), (128, 128), (256, 128), (384, 128),
            (512, 128), (640, 128), (768, 128)]

    with TileContext(nc) as tc, nc.allow_low_precision("f32r kernel"):
        with contextlib.ExitStack() as ctx:
            cpool = ctx.enter_context(tc.tile_pool(name="const", bufs=1))
            keep = ctx.enter_context(tc.tile_pool(name="keep", bufs=1))
            xtp_p = ctx.enter_context(tc.tile_pool(name="xtp", bufs=2))
            stg_p = ctx.enter_context(tc.tile_pool(name="stg", bufs=2))
            pt_p = ctx.enter_context(tc.tile_pool(name="ptp", bufs=3))
            on_p = ctx.enter_context(tc.tile_pool(name="onp", bufs=2))
            ovs_p = ctx.enter_context(tc.tile_pool(name="ovsp", bufs=1))
            rr_p = ctx.enter_context(tc.tile_pool(name="rrp", bufs=1))
            ys_p = ctx.enter_context(tc.tile_pool(name="ysp", bufs=3))
            sps_p = ctx.enter_context(
                tc.tile_pool(name="sps", bufs=2, space="PSUM"))
            ov_p = ctx.enter_context(
                tc.tile_pool(name="ovp", bufs=1, space="PSUM"))
            sm_p = ctx.enter_context(
                tc.tile_pool(name="smp", bufs=2, space="PSUM"))

            ident_f = cpool.tile([128, 128], f32)
            make_identity(nc, ident_f[:])
            ident_b = cpool.tile([128, 128], bf16)
            nc.vector.tensor_copy(ident_b[:], ident_f[:])
            mask_b = cpool.tile([128, 896], bf16)
            mask = cpool.tile([128, 896], f32)
            nc.gpsimd.memset(mask[:], 1.0)
            nc.gpsimd.affine_select(
                out=mask[:], in_=mask[:], compare_op=mybir.AluOpType.is_ge,
                fill=0.0, base=-384, channel_multiplier=-1, pattern=[[1, 896]])
            ones_t = cpool.tile([128, 64], f32)
            nc.gpsimd.memset(ones_t[:], 1.0)
            nc.vector.tensor_copy(mask_b[:], mask[:])
            ones_r = cpool.tile([128, 64], f32r)
            nc.vector.tensor_copy(ones_r[:], ones_t[:])
            # persistent reciprocal staging tiles: denominator half-rows land
            # at partitions {0,32,64,96}; the reciprocal runs over the full
            # contiguous tile (partition-parallel, so same cost) and the 1.0
            # filler rows are reciprocal-stable
            ones_s = cpool.tile([128, 256], f32)
            nc.gpsimd.memset(ones_s[:], 1.0)
            rr_ab = cpool.tile([128, 256], f32r)
            nc.vector.tensor_copy(rr_ab[:], ones_s[:])
            rr_c = cpool.tile([64, 256], f32r)
            nc.vector.tensor_copy(rr_c[:], ones_s[0:64, :])
            rr_b2 = cpool.tile([1, 256], f32r)

            bq_sb = cpool.tile([128, 7], f32)
            nc.sync.dma_start(bq_sb[:], bq[:, :])
            wp_sb = [keep.tile([64, C], f32r, tag=f"wp{h}", name=f"wp{h}")
                     for h in range(HPC)]

            QT_AB = keep.tile([128, T], bf16, tag="qt_ab")
            KT_AB = keep.tile([128, T], bf16, tag="kt_ab")
            QT_C = keep.tile([128, T], bf16, tag="qt_c")
            KT_C = keep.tile([128, T], bf16, tag="kt_c")
            Vp = [keep.tile([128, NKT * VP_W], bf16, tag=f"vp{h}",
                            name=f"vp{h}") for h in range(HPC)]

            # per-cc weight tiles; DMAs interleaved with x^T chunks so the
            # first QKV matmul can start after ~2 transfers
            wq_sb = [keep.tile([128, WQJ], f32r, tag=f"wq{cc}",
                               name=f"wq{cc}") for cc in range(NCC)]

            xt_tiles = {}
            stage_tiles = {}

            def issue_xt_dma(t, with_wq=False):
                t0 = t * QB
                xt = xtp_p.tile([128, NCC, QB], f32r, tag="xt",
                                name=f"xt{t}")
                xt_tiles[t] = xt
                for cc in range(NCC):
                    nc.gpsimd.dma_start(
                        xt[:, cc, :],
                        xt_d[cc * 128:(cc + 1) * 128, t0:t0 + QB])
                    if with_wq:
                        nc.gpsimd.dma_start(
                            wq_sb[cc][:], wq[cc * 128:(cc + 1) * 128, :])

            def qkv_blk(t, blk):
                t0 = t * QB
                xt = xt_tiles[t]
                j0, m = JBLK[blk]
                qp = sm_p.tile([128, QB], f32, tag="small",
                               name=f"qp{t}_{blk}")
                for cc in range(NCC):
                    nc.tensor.matmul(
                        qp[0:m, :], wq_sb[cc][:, j0:j0 + m], xt[:, cc, :],
                        start=(cc == 0), stop=(cc == NCC - 1))
                if blk == 0:
                    dest = QT_AB[:, t0:t0 + QB]
                elif blk == 1:
                    dest = KT_AB[:, t0:t0 + QB]
                elif blk == 2:
                    dest = QT_C[:, t0:t0 + QB]
                elif blk == 3:
                    dest = KT_C[:, t0:t0 + QB]
                else:
                    st = stg_p.tile([128, QB], bf16, tag=f"stage{blk}",
                                    name=f"stage{t}_{blk}")
                    stage_tiles[(t, blk)] = st
                    dest = st[:]
                nc.vector.tensor_scalar_add(
                    dest, qp[0:m, :], bq_sb[0:m, blk:blk + 1])

            def v_transposes(t):
                for h in range(HPC):
                    src = stage_tiles.pop((t, 4 + h))
                    vtp = sm_p.tile([128, 4, VP_W], bf16, tag="small",
                                    name=f"vtp{t}_{h}")
                    for i in range(4):
                        nc.tensor.transpose(
                            vtp[:, i, :], src[:, i * 128:(i + 1) * 128],
                            ident_b[:])
                    kt0 = t * 4
                    vview = Vp[h][:].rearrange("p (kt w) -> p kt w", w=VP_W)
                    nc.vector.tensor_copy(vview[:, kt0:kt0 + 4, :], vtp[:])

            def s_pair(qb, sps, half, kt, kt_t, qt_t, rows):
                """S^T for one head-half into sps[:, half*QB:...]."""
                q0 = qb * QB
                r0, r1 = rows
                nc.tensor.matmul(
                    sps[:, half * QB:(half + 1) * QB],
                    kt_t[r0:r1, kt * 128:(kt + 1) * 128],
                    qt_t[r0:r1, q0:q0 + QB], start=True, stop=True)

            def mask_and_av(qb, sps_pt, half, kt, ov, start, stop, h):
                q0 = qb * QB
                m = kt * 128 - q0
                if 0 <= m < QB:
                    nc.vector.tensor_tensor(
                        out=sps_pt[:, half * QB:(half + 1) * QB],
                        in0=sps_pt[:, half * QB:(half + 1) * QB],
                        in1=mask_b[:, 384 - m:896 - m], op=mult)
                nc.tensor.matmul(
                    ov[:], Vp[h][:, kt * VP_W:(kt + 1) * VP_W],
                    sps_pt[:, half * QB:(half + 1) * QB],
                    start=start, stop=stop)

            def attention(tb, with_next):
                qb = tb
                q0 = qb * QB
                nkt = 4 * qb + 4
                units = ([("AB", kt) for kt in range(nkt)]
                         + [("C", s) for s in range(nkt // 2)])

                if with_next:
                    issue_xt_dma(tb + 1)

                ovA = ov_p.tile([128, QB], f32, tag="ovA", name=f"ovA{qb}")
                ovB = ov_p.tile([128, QB], f32, tag="ovB", name=f"ovB{qb}")
                ovs = [None] * HPC
                ov_of = {0: ovA, 1: ovB}

                def issue_S(u):
                    kind, k = u
                    sps = sps_p.tile([128, 1024], f32, tag="sps",
                                     name=f"s{kind}{qb}_{k}")
                    if kind == "AB":
                        s_pair(qb, sps, 0, k, KT_AB, QT_AB, (0, 64))
                        s_pair(qb, sps, 1, k, KT_AB, QT_AB, (64, 128))
                    else:
                        s_pair(qb, sps, 0, 2 * k, KT_C, QT_C, (0, 64))
                        s_pair(qb, sps, 1, 2 * k + 1, KT_C, QT_C, (64, 128))
                    pt = pt_p.tile([128, 1024], bf16, tag="pt",
                                   name=f"pt{kind}{qb}_{k}")
                    nc.scalar.activation(pt[:], sps[:], Exp, scale=0.125)
                    return pt

                def issue_AV(u, pt):
                    kind, k = u
                    if kind == "AB":
                        mask_and_av(qb, pt, 0, k, ovA, k == 0,
                                    k == nkt - 1, 0)
                        mask_and_av(qb, pt, 1, k, ovB, k == 0,
                                    k == nkt - 1, 1)
                    else:
                        mask_and_av(qb, pt, 0, 2 * k, ov_of[2], k == 0,
                                    False, 2)
                        mask_and_av(qb, pt, 1, 2 * k + 1, ov_of[2], False,
                                    k == nkt // 2 - 1, 2)

                rr = [None] * HPC

                def evac_one(h):
                    """Evacuate O rows 0..64 (incl. denominator) to SBUF —
                    one copy frees the PSUM bank — then reciprocal from
                    SBUF off the critical path."""
                    ovs[h] = ovs_p.tile([65, QB], f32, tag=f"ovs{h}",
                                        name=f"ovs{qb}_{h}")
                    nc.vector.tensor_copy(ovs[h][:], ov_of[h][0:65, :])
                    rr[h] = rr_p.tile([65, QB], f32r, tag=f"rr{h}",
                                      name=f"rr{qb}_{h}")
                    nc.vector.reciprocal(rr[h][64:65, :], ovs[h][64:65, :])

                def evac_ab():
                    evac_one(0)
                    evac_one(1)

                def evac_c():
                    evac_one(2)

                def rbp_mult(h, rbp_t, col):
                    """Broadcast 1/denom to 64 rows (PE, into spare sps-ring
                    PSUM) and normalize on DVE."""
                    nc.tensor.matmul(rbp_t[0:64, col:col + QB],
                                     ones_r[64:65, :], rr[h][64:65, :],
                                     start=True, stop=True)
                    on = on_p.tile([64, QB], f32r, tag=f"on{h}",
                                   name=f"on{qb}_{h}")
                    nc.vector.tensor_tensor(out=on[:], in0=ovs[h][0:64, :],
                                            in1=rbp_t[0:64, col:col + QB],
                                            op=mult)
                    return on

                pending = None
                for i, u in enumerate(units):
                    if u == ("C", 0):
                        # reuses ovA's PSUM bank; safe because ovA is
                        # evacuated to SBUF right after its last AV below
                        ov_of[2] = ov_p.tile([128, QB], f32, tag="ovA",
                                             name=f"ovC{qb}")
                    pt = issue_S(u)
                    if pending is not None:
                        issue_AV(*pending)
                        if pending[0] == ("AB", nkt - 1):
                            evac_ab()
                    pending = (u, pt)

                # epilogue: interleave next t-block's QKV with normalize+proj
                issue_AV(*pending)                 # last C unit; ovC stop
                evac_c()
                if with_next:
                    qkv_blk(tb + 1, 0)
                rbp_ab_t = sps_p.tile([128, 1024], f32, tag="sps",
                                      name=f"rbpab{qb}")
                ons = [rbp_mult(0, rbp_ab_t, 0),
                       rbp_mult(1, rbp_ab_t, QB)]
                if with_next:
                    qkv_blk(tb + 1, 1)
                    qkv_blk(tb + 1, 2)
                rbp_c_t = sps_p.tile([128, 1024], f32, tag="sps",
                                     name=f"rbpc{qb}")
                ons.append(rbp_mult(2, rbp_c_t, 0))
                if with_next:
                    qkv_blk(tb + 1, 3)

                for co in range(NCC):
                    yp = sm_p.tile([128, QB], f32, tag="small",
                                   name=f"yp{qb}_{co}")
                    for h in range(HPC):
                        nc.tensor.matmul(
                            yp[:], wp_sb[h][:, co * 128:(co + 1) * 128],
                            ons[h][:], start=(h == 0), stop=(h == HPC - 1))
                    ys = ys_p.tile([128, QB], f32, tag="ys",
                                   name=f"ys{qb}_{co}")
                    nc.vector.tensor_copy(ys[:], yp[:])
                    nc.sync.dma_start(
                        y[co * 128:(co + 1) * 128, q0:q0 + QB], ys[:])

                if with_next:
                    for blk in (4, 5, 6):
                        qkv_blk(tb + 1, blk)
                    v_transposes(tb + 1)

            # prologue: t-block 0's QKV, then the pipelined attention loop
            issue_xt_dma(0, with_wq=True)
            for h in range(HPC):
                nc.gpsimd.dma_start(wp_sb[h][:], wp[h * 64:(h + 1) * 64, :])
            for blk in range(7):
                qkv_blk(0, blk)
            v_transposes(0)
            for tb in range(NTB):
                attention(tb, with_next=(tb + 1 < NTB))

    nc.finalize()
    return nc


def _core_inputs(x, w_attn, b_attn, w_proj):
    """Build the 8 per-core input maps (numpy float32)."""
    maps = []
    zc = np.zeros((C, 64), np.float32)
    for core in range(NCORES):
        b = core // 4
        heads = [HPC * (core % 4) + k for k in range(HPC)]
        hA, hB, hC = heads
        qc = lambda h: slice(h * HS, (h + 1) * HS)
        kc = lambda h: slice(C + h * HS, C + (h + 1) * HS)
        vc = lambda h: slice(2 * C + h * HS, 2 * C + (h + 1) * HS)
        wqm = np.concatenate([
            w_attn[:, qc(hA)], w_attn[:, qc(hB)],
            w_attn[:, kc(hA)], w_attn[:, kc(hB)],
            w_attn[:, qc(hC)], w_attn[:, qc(hC)],
            w_attn[:, kc(hC)], w_attn[:, kc(hC)],
            w_attn[:, vc(hA)], zc, w_attn[:, vc(hB)], zc,
            w_attn[:, vc(hC)], zc,
        ], axis=1)
        bqm = np.zeros((128, 7), np.float32)
        bqm[0:64, 0] = b_attn[qc(hA)]
        bqm[64:128, 0] = b_attn[qc(hB)]
        bqm[0:64, 1] = b_attn[kc(hA)]
        bqm[64:128, 1] = b_attn[kc(hB)]
        bqm[0:64, 2] = b_attn[qc(hC)]
        bqm[64:128, 2] = b_attn[qc(hC)]
        bqm[0:64, 3] = b_attn[kc(hC)]
        bqm[64:128, 3] = b_attn[kc(hC)]
        for i, h in enumerate(heads):
            bqm[0:64, 4 + i] = b_attn[vc(h)]
            bqm[64, 4 + i] = 1.0
        wpm = np.concatenate([w_proj[h * HS:(h + 1) * HS, :] for h in heads],
                             axis=0)
        maps.append({
            "xt": np.ascontiguousarray(x[b].T, np.float32),
            "wq": np.ascontiguousarray(wqm, np.float32),
            "bq": np.ascontiguousarray(bqm, np.float32),
            "wp": np.ascontiguousarray(wpm, np.float32),
        })
    return maps


def run_cores(in_maps, trace=False):
    from concourse import bass_utils
    if "nc" not in _CACHE:
        _CACHE["nc"] = _build()
    return bass_utils.run_bass_kernel_spmd(
        _CACHE["nc"], in_maps, list(range(NCORES)), trace=trace)


def kernel(x, w_attn, b_attn, w_proj, b_proj):
    x = np.asarray(x, np.float32)
    w_attn = np.asarray(w_attn, np.float32)
    b_attn = np.asarray(b_attn, np.float32)
    w_proj = np.asarray(w_proj, np.float32)
    b_proj = np.asarray(b_proj, np.float32)

    in_maps = _core_inputs(x, w_attn, b_attn, w_proj)
    res = run_cores(in_maps)
    y = np.zeros((B, T, C), np.float32)
    for b in range(B):
        acc = np.zeros((C, T), np.float64)
        for core in range(4 * b, 4 * b + 4):
            acc += res.results[core]["y"].astype(np.float64)
        y[b] = acc.T + b_proj[None, :]
    return y

